# revision 5
# baseline (speedup 1.0000x reference)
"""ExtSummModel Trainium2 Bass kernel (8-core data-parallel over batch).

Contract: kernel(**inputs) takes FULL numpy inputs (as produced by
reference.setup_inputs()) and returns the FULL [32, 256] float32 logits.

Self-contained: hardcodes all shapes. Shards batch 4 docs/core across 8 cores.
"""
import os
from contextlib import ExitStack

import numpy as np

import concourse.bass as bass
import concourse.mybir as mybir
import concourse.tile as tile
from concourse import bacc
from concourse.bass import AP
from concourse.bass_utils import run_bass_kernel_spmd
from concourse.masks import make_identity

F32 = mybir.dt.float32
I32 = mybir.dt.int32
AF = mybir.ActivationFunctionType
ALU = mybir.AluOpType

B, S, W = 32, 256, 32
T = 16
VOCAB, E, H, D = 400001, 300, 128, 128
NCORE = 8
BL = B // NCORE            # 4 docs per core
NCHUNK = 8                 # sentence chunks of 32 steps x 4 docs = 128 sents
CH = S // NCHUNK           # 32 steps per chunk
GATHER_ORDER = [0, 7, 1, 6, 2, 5, 3, 4]

LAST_EXEC_NS = None


def _ap(base: AP, extra_offset: int, free_dims):
    """Raw AP on the same tensor: keep partition dim, replace free dims."""
    return AP(base.tensor, base.offset + extra_offset, [base.ap[0]] + [list(d) for d in free_dims])


def build_nc(debug=False):
    nc = bacc.Bacc("TRN2", target_bir_lowering=False, debug=False, num_devices=NCORE)

    # ---- DRAM parameters ----
    table = nc.dram_tensor("table", [VOCAB, E], F32, kind="ExternalInput")
    widx_d = nc.dram_tensor("widx", [128, NCHUNK * CH], I32, kind="ExternalInput")
    wihT_d = {d: nc.dram_tensor(f"wihT_{d}", [3, 128, 384], F32, kind="ExternalInput") for d in "fb"}
    whhT_d = {d: nc.dram_tensor(f"whhT_{d}", [128, 384], F32, kind="ExternalInput") for d in "fb"}
    bhh_n_d = nc.dram_tensor("bhh_n2", [128, 2], F32, kind="ExternalInput")
    watt_d = nc.dram_tensor("watt", [512, 512], F32, kind="ExternalInput")
    vatt_d = nc.dram_tensor("vatt", [512], F32, kind="ExternalInput")
    w1T_d = nc.dram_tensor("w1T", [4, 128, 128], F32, kind="ExternalInput")
    b1_d = nc.dram_tensor("b1", [128], F32, kind="ExternalInput")
    w2T_d = nc.dram_tensor("w2T", [128], F32, kind="ExternalInput")
    b2_d = nc.dram_tensor("b2", [1], F32, kind="ExternalInput")
    dT_d = nc.dram_tensor("dT", [BL, 2, 2, 128, T], F32, kind="ExternalInput")
    oT_d = nc.dram_tensor("oT", [BL, T, S], F32, kind="ExternalInput")
    sel_d = nc.dram_tensor("sel", [64, 2 * BL], F32, kind="ExternalInput")

    out_d = nc.dram_tensor("logits", [BL, S], F32, kind="ExternalOutput")
    dbg = {}
    if debug:
        dbg["sr"] = nc.dram_tensor("dbg_sr", [128, 2 * BL * S], F32, kind="ExternalOutput")
        dbg["ssum"] = nc.dram_tensor("dbg_ssum", [128, NCHUNK * E], F32, kind="ExternalOutput")
        dbg["girz"] = nc.dram_tensor("dbg_girz", [128, S * 16], F32, kind="ExternalOutput")

    # collective buffers
    cc_in_d = nc.dram_tensor("cc_in", [2 * BL, 128], F32)
    cc_out_d = nc.dram_tensor("cc_out", [NCORE * 2 * BL, 128], F32, addr_space="Shared")

    with tile.TileContext(nc) as tc, ExitStack() as ctx:
        persist = ctx.enter_context(tc.tile_pool(name="persist", bufs=1))
        consts = ctx.enter_context(tc.tile_pool(name="consts", bufs=1))

        # ---- constants / weights to SBUF ----
        idx_sb = persist.tile([128, NCHUNK * CH], I32, name="idx_sb")
        nc.sync.dma_start(out=idx_sb[:, :], in_=widx_d[:, :])

        wihT = {}
        whhT = {}
        for d in "fb":
            for k in range(3):
                t_ = consts.tile([128, 384], F32, name=f"wihT_{d}{k}")
                nc.sync.dma_start(out=t_[:, :], in_=wihT_d[d][k, :, :])
                wihT[(d, k)] = t_
            t_ = consts.tile([128, 384], F32, name=f"whhT_{d}")
            nc.sync.dma_start(out=t_[:, :], in_=whhT_d[d][:, :])
            whhT[d] = t_
        bhh_n = consts.tile([128, 2], F32, name="bhh_n")
        nc.sync.dma_start(out=bhh_n[:, :], in_=bhh_n_d[:, :])
        watt = []
        for k in range(4):
            t_ = consts.tile([128, 512], F32, name=f"watt{k}")
            nc.sync.dma_start(out=t_[:, :], in_=watt_d[k * 128:(k + 1) * 128, :])
            watt.append(t_)
        v_sb = consts.tile([128, 4], F32, name="v_sb")
        nc.sync.dma_start(out=v_sb[:, :], in_=vatt_d.rearrange("(k p) -> p k", p=128))
        w1T = []
        for k in range(4):
            t_ = consts.tile([128, 128], F32, name=f"w1T{k}")
            nc.sync.dma_start(out=t_[:, :], in_=w1T_d[k, :, :])
            w1T.append(t_)
        b1_sb = consts.tile([128, 1], F32, name="b1_sb")
        nc.sync.dma_start(out=b1_sb[:, :], in_=b1_d.rearrange("(p one) -> p one", one=1))
        w2T_sb = consts.tile([128, 1], F32, name="w2T_sb")
        nc.sync.dma_start(out=w2T_sb[:, :], in_=w2T_d.rearrange("(p one) -> p one", one=1))
        b2_sb = consts.tile([1, 1], F32, name="b2_sb")
        nc.sync.dma_start(out=b2_sb[:, :], in_=b2_d.rearrange("(one x) -> one x", one=1))
        dT_sb = consts.tile([128, BL * 2 * 2 * T], F32, name="dT_sb")
        nc.sync.dma_start(
            out=dT_sb[:, :].rearrange("p (b d k t) -> p b d k t", b=BL, d=2, k=2),
            in_=dT_d.rearrange("b d k j t -> j b d k t"),
        )
        oT_sb = consts.tile([T, BL * S], F32, name="oT_sb")
        nc.sync.dma_start(
            out=oT_sb[:, :].rearrange("p (b s) -> p b s", b=BL),
            in_=oT_d.rearrange("b t s -> t b s"),
        )
        sel_sb = consts.tile([64, 2 * BL], F32, name="sel_sb")
        nc.sync.dma_start(out=sel_sb[:, :], in_=sel_d[:, :])

        ident = consts.tile([128, 128], F32, name="ident")
        make_identity(nc, ident[:, :])
        zeros8 = consts.tile([128, 8], F32, name="zeros8")
        nc.vector.memset(zeros8[:, :], 0.0)
        ones_row = consts.tile([1, 128], F32, name="ones_row")
        nc.vector.memset(ones_row[:, :], 1.0)
        ones_big = consts.tile([128, S], F32, name="ones_big")
        nc.vector.memset(ones_big[:, :], 1.0)

        # ---- persistent big buffers ----
        # sr: recurrence outputs. col = dir*1024 + b*256 + s
        sr = persist.tile([128, 2 * BL * S], F32, name="sr")
        # gi tiles per step-group g (32 steps): col = (i%32)*16 + gate4 + b
        gi_rz = [persist.tile([128, CH * 16], F32, name=f"gi_rz{g}") for g in range(NCHUNK)]
        gi_n = [persist.tile([128, CH * 8], F32, name=f"gi_n{g}") for g in range(NCHUNK)]

        # =========================================================
        # Phase 1: embedding gather + word-sum (CCE accumulate)
        # =========================================================
        gpool = ctx.enter_context(tc.tile_pool(name="gpool", bufs=1))
        ssum = [gpool.tile([128, E + 4], F32, name=f"ssum{c}", tag=f"ssum{c}") for c in range(NCHUNK)]
        for c in range(NCHUNK):
            nc.vector.memset(ssum[c][:, E:E + 4], 1.0)
        # interleave chains across chunks so each chain sticks to one SWDGE lane
        for w in range(W):
            for c in GATHER_ORDER:
                col = c * W + w
                nc.gpsimd.indirect_dma_start(
                    out=ssum[c][:, 0:E],
                    out_offset=None,
                    in_=table[:, :],
                    in_offset=bass.IndirectOffsetOnAxis(ap=idx_sb[:, col:col + 1], axis=0),
                    compute_op=(ALU.add if w > 0 else ALU.bypass),
                )

        # =========================================================
        # Phase 2: per-chunk transpose -> sentT[c][k]  [128, 128]
        #   rows of k=2 tile: 0:44 = E 256:300, row 44 = 1.0 (bias), rest 0
        # =========================================================
        sentT = [[None] * 3 for _ in range(NCHUNK)]
        ctxA = ExitStack()
        tp_psum = ctxA.enter_context(tc.tile_pool(name="tp_psum", bufs=2, space="PSUM"))
        stp = ctx.enter_context(tc.tile_pool(name="stp", bufs=1))
        for c in range(NCHUNK):
            for k in range(3):
                t_ = stp.tile([128, 128], F32, name=f"sentT{c}_{k}", tag=f"sentT{c}_{k}")
                sentT[c][k] = t_
        for c in GATHER_ORDER:
            for k in range(3):
                ecnt = 128 if k < 2 else E + 1 - 256
                pt = tp_psum.tile([128, 128], F32, name=f"tp{c}{k}", tag="tp")
                nc.tensor.transpose(
                    out=pt[:ecnt, :], in_=ssum[c][:, k * 128:k * 128 + ecnt], identity=ident[:, :]
                )
                if k == 2:
                    nc.vector.memset(sentT[c][2][:, :], 0.0)
                nc.vector.tensor_copy(sentT[c][k][:ecnt, :], pt[:ecnt, :])

        # =========================================================
        # Phase 3: gi matmuls.
        #   gi^T[gate_chunk m] [128, (b,ds)] = sum_k wihT[d][k][:, m*128:...]^T @ sentT[c][k]
        #   -> interleaved into gi_rz[g] / gi_n[g]
        # =========================================================
        gi_psum = ctxA.enter_context(tc.tile_pool(name="gi_psum", bufs=2, space="PSUM"))
        for c in GATHER_ORDER:
            for di, d in enumerate("fb"):
                g = c if d == "f" else NCHUNK - 1 - c
                for m in range(3):
                    pg = gi_psum.tile([128, 128], F32, name=f"gip{c}{d}{m}", tag="gip")
                    for k in range(3):
                        nc.tensor.matmul(
                            pg[:, :],
                            wihT[(d, k)][:, m * 128:(m + 1) * 128],
                            sentT[c][k][:, :],
                            start=(k == 0), stop=(k == 2),
                        )
                    # copy out with interleave: psum free iter = (b outer, ds inner)
                    if m < 2:
                        dst_t = gi_rz[g]
                        if d == "f":
                            dst = _ap(dst_t[:, :], m * 4, [[1, 4], [16, CH]])
                        else:
                            dst = _ap(dst_t[:, :], (CH - 1) * 16 + 8 + m * 4, [[1, 4], [-16, CH]])
                    else:
                        dst_t = gi_n[g]
                        if d == "f":
                            dst = _ap(dst_t[:, :], 0, [[1, 4], [8, CH]])
                        else:
                            dst = _ap(dst_t[:, :], (CH - 1) * 8 + 4, [[1, 4], [-8, CH]])
                    src = _ap(pg[:, :], 0, [[CH, 4], [1, CH]])
                    nc.scalar.activation(dst, src, AF.Copy, bias=0.0)

        # =========================================================
        # Phase 4: bi-GRU recurrence, 256 steps.
        # layout: [128 hidden, 8] = [fwd b0..3, bwd b0..3]
        # =========================================================
        rec_ps = ctxA.enter_context(tc.tile_pool(name="rec_ps", bufs=2, space="PSUM"))
        rec_sb = ctx.enter_context(tc.tile_pool(name="rec_sb", bufs=3))
        sr_base = sr[:, :]
        for i in range(S):
            g, j = i // CH, i % CH
            p_rz = rec_ps.tile([128, 16], F32, name=f"prz{i}", tag="prz")
            p_n = rec_ps.tile([128, 8], F32, name=f"pn{i}", tag="pn")
            if i == 0:
                h_f = zeros8[:, 0:4]
                h_b = zeros8[:, 4:8]
                h_fb = zeros8[:, :].rearrange("p (d b) -> p d b", d=2)
            else:
                h_f = _ap(sr_base, i - 1, [[256, 4]])
                h_b = _ap(sr_base, 1024 + 256 - i, [[256, 4]])
                h_fb = _ap(sr_base, i - 1, [[1281 - 2 * i, 2], [256, 4]])
            nc.tensor.matmul(p_rz[:, 0:4], whhT["f"][:, 0:128], h_f, start=True, stop=True)
            nc.tensor.matmul(p_rz[:, 4:8], whhT["f"][:, 128:256], h_f, start=True, stop=True)
            nc.tensor.matmul(p_rz[:, 8:12], whhT["b"][:, 0:128], h_b, start=True, stop=True)
            nc.tensor.matmul(p_rz[:, 12:16], whhT["b"][:, 128:256], h_b, start=True, stop=True)
            nc.tensor.matmul(p_n[:, 0:4], whhT["f"][:, 256:384], h_f, start=True, stop=True)
            nc.tensor.matmul(p_n[:, 4:8], whhT["b"][:, 256:384], h_b, start=True, stop=True)

            t_rz = rec_sb.tile([128, 16], F32, name=f"trz{i}", tag="trz")
            nc.vector.tensor_add(t_rz[:, :], p_rz[:, :], gi_rz[g][:, j * 16:(j + 1) * 16])
            rz = rec_sb.tile([128, 16], F32, name=f"rz{i}", tag="rz")
            nc.scalar.activation(rz[:, :], t_rz[:, :], AF.Sigmoid)

            t_n1 = rec_sb.tile([128, 8], F32, name=f"tn1_{i}", tag="tn1")
            nc.vector.tensor_add(
                t_n1[:, :].rearrange("p (d b) -> p d b", d=2),
                p_n[:, :].rearrange("p (d b) -> p d b", d=2),
                _ap(bhh_n[:, :], 0, [[1, 2], [0, 4]]),
            )
            t_n2 = rec_sb.tile([128, 8], F32, name=f"tn2_{i}", tag="tn2")
            r_sl = _ap(rz[:, :], 0, [[8, 2], [1, 4]])
            nc.vector.tensor_mul(t_n2[:, :].rearrange("p (d b) -> p d b", d=2), t_n1[:, :].rearrange("p (d b) -> p d b", d=2), r_sl)
            t_n3 = rec_sb.tile([128, 8], F32, name=f"tn3_{i}", tag="tn3")
            nc.vector.tensor_add(t_n3[:, :], t_n2[:, :], gi_n[g][:, j * 8:(j + 1) * 8])
            n_sb = rec_sb.tile([128, 8], F32, name=f"nsb{i}", tag="nsb")
            nc.scalar.activation(n_sb[:, :], t_n3[:, :], AF.Tanh)

            t_d = rec_sb.tile([128, 8], F32, name=f"td{i}", tag="td")
            nc.vector.tensor_sub(t_d[:, :].rearrange("p (d b) -> p d b", d=2), h_fb, n_sb[:, :].rearrange("p (d b) -> p d b", d=2))
            t_e = rec_sb.tile([128, 8], F32, name=f"te{i}", tag="te")
            z_sl = _ap(rz[:, :], 4, [[8, 2], [1, 4]])
            nc.vector.tensor_mul(t_e[:, :].rearrange("p (d b) -> p d b", d=2), t_d[:, :].rearrange("p (d b) -> p d b", d=2), z_sl)
            h_out = _ap(sr_base, i, [[1279 - 2 * i, 2], [256, 4]])
            nc.vector.tensor_add(h_out, n_sb[:, :].rearrange("p (d b) -> p d b", d=2), t_e[:, :].rearrange("p (d b) -> p d b", d=2))

        ctxA.close()

        # =========================================================
        # Phase 5: collective for doc_vec pieces
        # =========================================================
        cc_sb = persist.tile([128, 2 * BL], F32, name="cc_sb")
        nc.vector.tensor_copy(cc_sb[:, 0:BL], _ap(sr_base, 255, [[256, BL]]))
        nc.vector.tensor_copy(cc_sb[:, BL:2 * BL], _ap(sr_base, 1024, [[256, BL]]))
        nc.sync.dma_start(
            out=AP(cc_in_d, 0, [[1, 128], [128, 2 * BL]]),
            in_=cc_sb[:, :],
        )
        nc.gpsimd.collective_compute(
            "AllGather",
            ALU.bypass,
            replica_groups=[list(range(NCORE))],
            ins=[cc_in_d[:, :]],
            outs=[cc_out_d[:, :]],
        )
        pieces_sb = persist.tile([64, 128], F32, name="pieces_sb")
        nc.sync.dma_start(out=pieces_sb[:, :], in_=cc_out_d[:, :])
        ctxB = ExitStack()
        dv_ps = ctxB.enter_context(tc.tile_pool(name="dv_ps", bufs=1, space="PSUM"))
        dv_psum = dv_ps.tile([128, 2 * BL], F32, name="dv_psum")
        nc.tensor.matmul(dv_psum[:, :], pieces_sb[:, :], sel_sb[:, :], start=True, stop=True)
        dv_sb = persist.tile([128, 2 * BL], F32, name="dv_sb")
        nc.vector.tensor_copy(dv_sb[:, :], dv_psum[:, :])

        # =========================================================
        # Phase 6: topic representations
        # =========================================================
        tpc_ps = ctxB.enter_context(tc.tile_pool(name="tpc_ps", bufs=2, space="PSUM"))
        tpc_sb = ctx.enter_context(tc.tile_pool(name="tpc_sb", bufs=1))
        trT = [[None, None] for _ in range(BL)]
        for b in range(BL):
            for di, d in enumerate("fb"):
                # padF k-tiles = transposes of sr slices
                pads = []
                for k in range(2):
                    pt = tpc_ps.tile([128, 128], F32, name=f"padp{b}{d}{k}", tag="padp")
                    nc.tensor.transpose(
                        out=pt[:, :],
                        in_=sr[:, di * 1024 + b * 256 + k * 128: di * 1024 + b * 256 + (k + 1) * 128],
                        identity=ident[:, :],
                    )
                    ps = tpc_sb.tile([128, 128], F32, name=f"pads{b}{d}{k}", tag=f"pads{b}{d}{k}")
                    nc.vector.tensor_copy(ps[:, :], pt[:, :])
                    pads.append(ps)
                tm_ps = tpc_ps.tile([T, 128], F32, name=f"tmps{b}{d}", tag="tmps", bufs=1)
                for k in range(2):
                    base = ((b * 2 + di) * 2 + k) * T
                    nc.tensor.matmul(
                        tm_ps[:, :], dT_sb[:, base:base + T], pads[k][:, :],
                        start=(k == 0), stop=(k == 1),
                    )
                tm_sb = tpc_sb.tile([T, 128], F32, name=f"tmsb{b}{d}", tag=f"tmsb{b}{d}")
                nc.vector.tensor_copy(tm_sb[:, :], tm_ps[:, :])
                tr_ps = tpc_ps.tile([128, S], F32, name=f"trps{b}{d}", tag="trps")
                nc.tensor.matmul(tr_ps[:, :], tm_sb[:, :], oT_sb[:, b * S:(b + 1) * S], start=True, stop=True)
                t_ = tpc_sb.tile([128, S], F32, name=f"trT{b}{d}", tag=f"trT{b}{d}")
                nc.vector.tensor_copy(t_[:, :], tr_ps[:, :])
                trT[b][di] = t_

        # doc_rep^T tiles
        docT = [[None, None] for _ in range(BL)]
        for b in range(BL):
            for half in range(2):
                t_ = tpc_sb.tile([128, S], F32, name=f"docT{b}{half}", tag=f"docT{b}{half}")
                nc.vector.tensor_scalar_mul(t_[:, :], ones_big[:, :], dv_sb[:, 2 * b + half:2 * b + half + 1])
                docT[b][half] = t_

        # =========================================================
        # Phase 7: attention scores + softmax + context
        # =========================================================
        ctxB.close()
        ctxC = ExitStack()
        att_ps = ctxC.enter_context(tc.tile_pool(name="att_ps", bufs=2, space="PSUM"))
        att_sb = ctx.enter_context(tc.tile_pool(name="att_sb", bufs=2))
        ctxT = [[None, None] for _ in range(BL)]
        for b in range(BL):
            srT = [sr[:, b * 256:(b + 1) * 256], sr[:, 1024 + b * 256: 1024 + (b + 1) * 256]]
            wtiles = {}
            for ti, typ in enumerate(("ds", "ts")):
                kt = (docT[b] if typ == "ds" else trT[b]) + srT
                sc_ps = att_ps.tile([1, S], F32, name=f"scps{b}{typ}", tag="scps")
                for m in range(4):
                    pA = att_ps.tile([128, S], F32, name=f"pA{b}{typ}{m}", tag="pA")
                    for k in range(4):
                        nc.tensor.matmul(
                            pA[:, :], watt[k][:, m * 128:(m + 1) * 128], kt[k],
                            start=(k == 0), stop=(k == 3),
                        )
                    a_sb = att_sb.tile([128, S], F32, name=f"asb{b}{typ}{m}", tag="asb")
                    nc.scalar.activation(a_sb[:, :], pA[:, :], AF.Tanh)
                    nc.tensor.matmul(
                        sc_ps[:, :], v_sb[:, m:m + 1], a_sb[:, :],
                        start=(m == 0), stop=(m == 3),
                    )
                # softmax over S on [1, S]
                mx = att_sb.tile([1, 1], F32, name=f"mx{b}{typ}", tag="mx")
                nc.vector.tensor_reduce(mx[:, :], sc_ps[:, :], axis=mybir.AxisListType.X, op=ALU.max)
                nmx = att_sb.tile([1, 1], F32, name=f"nmx{b}{typ}", tag="nmx")
                nc.vector.tensor_scalar_mul(nmx[:, :], mx[:, :], -1.0)
                ex = att_sb.tile([1, S], F32, name=f"ex{b}{typ}", tag="ex")
                nc.scalar.activation(ex[:, :], sc_ps[:, :], AF.Exp, bias=nmx[:, :])
                sm = att_sb.tile([1, 1], F32, name=f"sm{b}{typ}", tag="sm")
                nc.vector.tensor_reduce(sm[:, :], ex[:, :], axis=mybir.AxisListType.X, op=ALU.add)
                rs = att_sb.tile([1, 1], F32, name=f"rs{b}{typ}", tag="rs")
                nc.vector.reciprocal(rs[:, :], sm[:, :])
                wt = att_sb.tile([1, S], F32, name=f"wt{b}{typ}", tag="wt")
                nc.vector.tensor_scalar_mul(wt[:, :], ex[:, :], rs[:, :])
                wtiles[typ] = wt
            # context^T = bcast(w_ds) * docT + bcast(w_ts) * trT
            for half in range(2):
                bc_ds = att_ps.tile([128, S], F32, name=f"bcds{b}{half}", tag="bcds", bufs=1)
                nc.tensor.matmul(bc_ds[:, :], ones_row[:, :], wtiles["ds"][:, :], start=True, stop=True)
                bc_ts = att_ps.tile([128, S], F32, name=f"bcts{b}{half}", tag="bcts", bufs=1)
                nc.tensor.matmul(bc_ts[:, :], ones_row[:, :], wtiles["ts"][:, :], start=True, stop=True)
                t1 = att_sb.tile([128, S], F32, name=f"t1_{b}{half}", tag="t1")
                nc.vector.tensor_scalar_mul(t1[:, :], bc_ds[:, :], dv_sb[:, 2 * b + half:2 * b + half + 1])
                t2 = att_sb.tile([128, S], F32, name=f"t2_{b}{half}", tag="t2")
                nc.vector.tensor_mul(t2[:, :], bc_ts[:, :], trT[b][half][:, :])
                ct = tpc_sb.tile([128, S], F32, name=f"ctxT{b}{half}", tag=f"ctxT{b}{half}")
                nc.vector.tensor_add(ct[:, :], t1[:, :], t2[:, :])
                ctxT[b][half] = ct

        # =========================================================
        # Phase 8: MLP head
        # =========================================================
        ctxC.close()
        ctxD = ExitStack()
        mlp_ps = ctxD.enter_context(tc.tile_pool(name="mlp_ps", bufs=2, space="PSUM"))
        mlp_sb = ctx.enter_context(tc.tile_pool(name="mlp_sb", bufs=2))
        for b in range(BL):
            srT = [sr[:, b * 256:(b + 1) * 256], sr[:, 1024 + b * 256: 1024 + (b + 1) * 256]]
            kt = srT + ctxT[b]
            ph = mlp_ps.tile([128, S], F32, name=f"ph{b}", tag="ph")
            for k in range(4):
                nc.tensor.matmul(ph[:, :], w1T[k][:, :], kt[k], start=(k == 0), stop=(k == 3))
            hdd = mlp_sb.tile([128, S], F32, name=f"hdd{b}", tag="hdd")
            nc.scalar.activation(hdd[:, :], ph[:, :], AF.Relu, bias=b1_sb[:, :])
            pl = mlp_ps.tile([1, S], F32, name=f"pl{b}", tag="pl")
            nc.tensor.matmul(pl[:, :], w2T_sb[:, :], hdd[:, :], start=True, stop=True)
            lo = mlp_sb.tile([1, S], F32, name=f"lo{b}", tag="lo")
            nc.vector.tensor_scalar_add(lo[:, :], pl[:, :], b2_sb[:, :])
            nc.sync.dma_start(out=out_d[b, :].rearrange("(one s) -> one s", one=1), in_=lo[:, :])

        ctxD.close()

        if debug:
            ob = persist.tile([128, NCHUNK * E], F32, name="dbg_ssum_sb")
            for c in range(NCHUNK):
                nc.vector.tensor_copy(ob[:, c * E:(c + 1) * E], ssum[c][:, 0:E])
            nc.sync.dma_start(out=dbg["ssum"][:, :], in_=ob[:, :])
            nc.sync.dma_start(out=dbg["sr"][:, :], in_=sr[:, :])
            gb = persist.tile([128, S * 16], F32, name="dbg_girz_sb")
            for g in range(NCHUNK):
                nc.vector.tensor_copy(gb[:, g * CH * 16:(g + 1) * CH * 16], gi_rz[g][:, :])
            nc.sync.dma_start(out=dbg["girz"][:, :], in_=gb[:, :])

    nc.compile()
    return nc


def prep_core_inputs(core, word_ids, topic_start_ends, embed_table,
                     Wih_f, Whh_f, bih_f, bhh_f, Wih_b, Whh_b, bih_b, bhh_b,
                     v_attention, W_attention, W1, b1, W2, b2):
    f32 = np.float32
    wid = np.ascontiguousarray(word_ids[core * BL:(core + 1) * BL]).astype(np.int32)
    tse = topic_start_ends[core * BL:(core + 1) * BL].astype(np.int64)

    # widx[p=b*32+ds, c*32+w]
    widx = wid.reshape(BL, NCHUNK, CH, W).transpose(0, 2, 1, 3).reshape(128, NCHUNK * W)
    widx = np.ascontiguousarray(widx)

    def mk_wihT(Wih, bih, bhh):
        Z = np.zeros((384, 384), f32)
        Z[:300, :] = (Wih.T / 32.0).astype(f32)
        bias = bih.astype(np.float64).copy()
        bias[:256] += bhh[:256].astype(np.float64)
        Z[300, :] = bias.astype(f32)
        return np.ascontiguousarray(Z.reshape(3, 128, 384))

    wihT_f = mk_wihT(Wih_f, bih_f, bhh_f)
    wihT_b = mk_wihT(Wih_b, bih_b, bhh_b)
    whhT_f = np.ascontiguousarray(Whh_f.T.astype(f32))
    whhT_b = np.ascontiguousarray(Whh_b.T.astype(f32))
    bhh_n2 = np.stack([bhh_f[256:384], bhh_b[256:384]], axis=1).astype(f32)

    # topic selection matrices
    dT = np.zeros((BL, 2, 2, 128, T), f32)
    starts = tse[:, :, 0]
    ends = tse[:, :, 1]

    def add_entry(b, d, j, t, val):
        j = int(j) % 258
        if j == 0 or j == 257:
            return
        if 1 <= j <= 128:
            dT[b, d, 0, j - 1, t] += val
        else:
            dT[b, d, 1, j - 129, t] += val

    for b in range(BL):
        for t in range(T):
            add_entry(b, 0, ends[b, t], t, 1.0)
            add_entry(b, 0, starts[b, t] - 1, t, -1.0)
            add_entry(b, 1, starts[b, t], t, 1.0)
            add_entry(b, 1, ends[b, t] + 1, t, -1.0)

    sidx = np.arange(S, dtype=np.int64)
    # topic_id = sum_t [s >= starts-1] - 1  (mod T on wrap)
    tid = (sidx[None, :, None] >= (starts - 1)[:, None, :]).sum(-1) - 1
    tid = tid % T
    oT = np.zeros((BL, T, S), f32)
    for b in range(BL):
        oT[b, tid[b], sidx] = 1.0

    sel = np.zeros((64, 2 * BL), f32)
    for bl in range(BL):
        g = core * BL + bl
        for half in range(2):
            if g < 16:
                d0 = 2 * g + half
                piece = (d0 // 4) * 8 + (d0 % 4)
            else:
                d0 = 2 * (g - 16) + half
                piece = (d0 // 4) * 8 + 4 + (d0 % 4)
            sel[piece, 2 * bl + half] = 1.0

    return {
        "table": embed_table,
        "widx": widx,
        "wihT_f": wihT_f, "wihT_b": wihT_b,
        "whhT_f": whhT_f, "whhT_b": whhT_b,
        "bhh_n2": bhh_n2,
        "watt": np.ascontiguousarray(W_attention.astype(f32)),
        "vatt": np.ascontiguousarray(v_attention.reshape(-1).astype(f32)),
        "w1T": np.ascontiguousarray(W1.T.astype(f32).reshape(4, 128, 128)),
        "b1": b1.astype(f32),
        "w2T": W2.reshape(-1).astype(f32),
        "b2": b2.astype(f32),
        "dT": dT, "oT": oT, "sel": sel,
    }


_NC_CACHE = {}


def kernel(**inputs):
    global LAST_EXEC_NS
    debug = bool(int(os.environ.get("EXTSUMM_DEBUG", "0")))
    trace = bool(int(os.environ.get("EXTSUMM_TRACE", "0")))
    key = debug
    if key not in _NC_CACHE:
        _NC_CACHE[key] = build_nc(debug=debug)
    nc = _NC_CACHE[key]
    in_maps = [prep_core_inputs(c, **inputs) for c in range(NCORE)]
    res = run_bass_kernel_spmd(nc, in_maps, core_ids=list(range(NCORE)), trace=trace)
    LAST_EXEC_NS = res.exec_time_ns
    kernel.last_results = res
    out = np.concatenate([res.results[c]["logits"] for c in range(NCORE)], axis=0)
    return out.astype(np.float32)
